# revision 3
# baseline (speedup 1.0000x reference)
"""Bahdanau additive attention on 8 TRN2 NeuronCores.

Problem shapes: encoder [4, 1024, 256], decoder [4, 256, 256],
W_a/U_a [256, 256], V_a [256, 1].
reference:
    enc_proj = enc @ W_a                  [B, E, H]
    dec_proj = dec @ U_a                  [B, D, H]
    score[b,d,e] = sum_h V[h] * tanh(dec_proj[b,d,h] + enc_proj[b,e,h])
    w = softmax(score, axis=-1)           [B, D, E]
    ctx = w @ enc                         [B, D, H]
    return (ctx, w)

Sharding: 8 cores = (batch b = core//2) x (decoder-row half = core%2).
Each core owns 128 decoder rows of one batch element; outputs are
disjoint so no collectives are needed.

Per-core dataflow (h lives on SBUF partitions, 2 chunks of 128):
  - TensorE: enc_projT[h,e] = W^T-blocks @ encT (psum-resident),
    dec_projT[h,d] likewise.
  - ScalarE: t[h,e] = tanh(enc_projT[h,e] + bias=dec_projT[h,d]) per
    decoder row d (the broadcast add is the free per-partition bias).
  - TensorE: score rows via matmul with a shifted-V trick: lhsT is a
    128-col slice of a [128, 256] tensor whose only nonzero column
    (at index 128) holds V; slice [128-d, 256-d) puts V in column d,
    so row d of PSUM accumulates score[d, :] while all other rows
    get += 0.  512 bf16 matmuls accumulate into one psum block
    score[128 d, 1024 e].
  - softmax along free axis (reduce_max -> exp with accum_out -> mul
    by reciprocal), transpose w via TensorE, context matmul against
    natural-layout enc, DMA out.

Mode v1: per-(d, chunk) biased tanh ACTIVATE reading enc_projT from
PSUM (256 instrs of FD=1024).
Mode v2: DVE tensor_scalar pre-add in bf16 (4x mode) into a wide
buffer, then one giant tanh ACTIVATE per group of G=8 decoder rows
(FD=16384) to amortize ScalarE per-instruction overhead.
"""

import os
import sys

for _p in (
    "/opt/trn_rl_repo",
    "/root/.axon_site",
    "/root/.axon_site/_ro/trn_rl_repo",
    "/root/.axon_site/_ro/pypackages",
):
    if os.path.isdir(_p) and _p not in sys.path:
        sys.path.append(_p)

import ml_dtypes
import numpy as np

import concourse.mybir as mybir
from concourse import bacc, bass, tile

F32 = mybir.dt.float32
BF16 = mybir.dt.bfloat16

B, T_ENC, T_DEC, H = 4, 1024, 256, 256
P = 128  # SBUF partitions
HC = H // P  # h chunks (2)
DPC = 128  # decoder rows per core
N_CORES = 8

MODE = os.environ.get("ATTN_KERNEL_MODE", "v2")


def build_graph(mode=MODE):
    nc = bacc.Bacc("TRN2", target_bir_lowering=False, debug=False)

    enc_nat_d = nc.declare_dram_parameter("enc_nat", [8, P, H], F32, isOutput=False)
    encT_d = nc.declare_dram_parameter("encT", [HC, P, T_ENC], F32, isOutput=False)
    decT_d = nc.declare_dram_parameter("decT", [HC, P, DPC], F32, isOutput=False)
    W_d = nc.declare_dram_parameter("W", [HC, HC, P, P], F32, isOutput=False)
    U_d = nc.declare_dram_parameter("U", [HC, HC, P, P], F32, isOutput=False)
    Vbig_d = nc.declare_dram_parameter("Vbig", [P, HC, 2 * P], BF16, isOutput=False)
    ident_d = nc.declare_dram_parameter("ident", [P, P], F32, isOutput=False)
    w_out_d = nc.declare_dram_parameter("w_out", [DPC, T_ENC], F32, isOutput=True)
    ctx_out_d = nc.declare_dram_parameter("ctx_out", [DPC, H], F32, isOutput=True)

    TANH = mybir.ActivationFunctionType.Tanh
    EXP = mybir.ActivationFunctionType.Exp

    with tile.TileContext(nc) as tc:
        with (
            tc.tile_pool(name="const", bufs=1) as cpool,
            tc.tile_pool(name="psum_big", bufs=1, space="PSUM") as pbig,
            tc.tile_pool(name="psum_aux", bufs=1, space="PSUM") as paux,
            tc.tile_pool(name="work", bufs=1) as wpool,
            tc.tile_pool(name="epi", bufs=1) as epool,
        ):
            # ---------------- constants in ----------------
            encT_sb = cpool.tile([P, HC, T_ENC], F32, tag="encT_sb")
            for c in range(HC):
                nc.sync.dma_start(out=encT_sb[:, c, :], in_=encT_d[c])
            W_sb = cpool.tile([P, HC, HC, P], F32, tag="W_sb")
            U_sb = cpool.tile([P, HC, HC, P], F32, tag="U_sb")
            for ci in range(HC):
                for co in range(HC):
                    nc.sync.dma_start(out=W_sb[:, ci, co, :], in_=W_d[ci, co])
                    nc.sync.dma_start(out=U_sb[:, ci, co, :], in_=U_d[ci, co])
            decT_sb = cpool.tile([P, HC, DPC], F32, tag="decT_sb")
            for c in range(HC):
                nc.sync.dma_start(out=decT_sb[:, c, :], in_=decT_d[c])
            Vbig_sb = cpool.tile([P, HC, 2 * P], BF16, tag="Vbig_sb")
            nc.sync.dma_start(out=Vbig_sb[:], in_=Vbig_d[:])
            ident_sb = cpool.tile([P, P], F32, tag="ident_sb")
            nc.sync.dma_start(out=ident_sb[:], in_=ident_d[:])
            enc_nat_sb = cpool.tile([P, 8, H], F32, tag="enc_nat_sb")
            for t in range(8):
                nc.sync.dma_start(out=enc_nat_sb[:, t, :], in_=enc_nat_d[t])

            # ---------------- projections ----------------
            # enc_projT[co][h, e] = sum_ci W[ci, co]^T-block @ encT[ci]
            encproj_ps = []
            for co in range(HC):
                ep = pbig.tile([P, T_ENC], F32, tag=f"encproj{co}", name=f"encproj{co}")
                encproj_ps.append(ep)
                for half in range(2):
                    sl = slice(half * 512, (half + 1) * 512)
                    for ci in range(HC):
                        nc.tensor.matmul(
                            ep[:, sl],
                            W_sb[:, ci, co, :],
                            encT_sb[:, ci, sl],
                            start=(ci == 0),
                            stop=(ci == HC - 1),
                        )
            decproj_ps = paux.tile([P, HC, DPC], F32, tag="aux", bufs=2, name="decproj_ps")
            for co in range(HC):
                for ci in range(HC):
                    nc.tensor.matmul(
                        decproj_ps[:, co, :],
                        U_sb[:, ci, co, :],
                        decT_sb[:, ci, :],
                        start=(ci == 0),
                        stop=(ci == HC - 1),
                    )
            decproj_sb = cpool.tile([P, HC, DPC], F32, tag="decproj_sb")
            nc.vector.tensor_copy(decproj_sb[:], decproj_ps[:])

            score_ps = pbig.tile([P, T_ENC], F32, tag="score", name="score")

            # ---------------- main loop: tanh + V-reduction ----------------
            if mode == "v1":
                for d in range(DPC):
                    th = wpool.tile(
                        [P, HC, T_ENC], BF16, tag="th", bufs=3, name=f"th{d}"
                    )
                    for c in range(HC):
                        nc.scalar.activation(
                            th[:, c, :],
                            encproj_ps[c][:],
                            TANH,
                            bias=decproj_sb[:, c, d : d + 1],
                        )
                    for c in range(HC):
                        for half in range(2):
                            sl = slice(half * 512, (half + 1) * 512)
                            nc.tensor.matmul(
                                score_ps[:, sl],
                                Vbig_sb[:, c, P - d : 2 * P - d],
                                th[:, c, sl],
                                start=(d == 0 and c == 0),
                                stop=(d == DPC - 1 and c == HC - 1),
                            )
            else:  # v2
                # bf16 copy of enc_projT in SBUF for the 4x DVE pre-add
                encproj_bf = cpool.tile([P, HC, T_ENC], BF16, tag="encproj_bf")
                for c in range(HC):
                    nc.vector.tensor_copy(encproj_bf[:, c, :], encproj_ps[c][:])
                G = 8
                for g in range(DPC // G):
                    pre = wpool.tile(
                        [P, G, HC, T_ENC], BF16, tag="pre", bufs=2, name=f"pre{g}"
                    )
                    for r in range(G):
                        d = g * G + r
                        for c in range(HC):
                            nc.vector.tensor_scalar_add(
                                pre[:, r, c, :],
                                encproj_bf[:, c, :],
                                decproj_sb[:, c, d : d + 1],
                            )
                    th = wpool.tile(
                        [P, G, HC, T_ENC], BF16, tag="th", bufs=2, name=f"th{g}"
                    )
                    nc.scalar.activation(th[:], pre[:], TANH)
                    for r in range(G):
                        d = g * G + r
                        for c in range(HC):
                            for half in range(2):
                                sl = slice(half * 512, (half + 1) * 512)
                                nc.tensor.matmul(
                                    score_ps[:, sl],
                                    Vbig_sb[:, c, P - d : 2 * P - d],
                                    th[:, r, c, sl],
                                    start=(d == 0 and c == 0),
                                    stop=(d == DPC - 1 and c == HC - 1),
                                )

            # ---------------- softmax ----------------
            negmax = epool.tile([P, 1], F32, tag="negmax")
            nc.vector.tensor_reduce(
                negmax[:],
                score_ps[:],
                mybir.AxisListType.X,
                mybir.AluOpType.max,
                negate=True,
            )
            expw = epool.tile([P, T_ENC], F32, tag="expw")
            sumexp = epool.tile([P, 1], F32, tag="sumexp")
            nc.scalar.activation(
                expw[:], score_ps[:], EXP, bias=negmax[:], accum_out=sumexp[:]
            )
            rec = epool.tile([P, 1], F32, tag="rec")
            nc.vector.reciprocal(rec[:], sumexp[:])
            wnorm = epool.tile([P, T_ENC], F32, tag="wnorm")
            nc.vector.tensor_scalar_mul(wnorm[:], expw[:], rec[:])
            nc.sync.dma_start(out=w_out_d[:], in_=wnorm[:])

            # ---------------- context = w @ enc ----------------
            wT_sb = epool.tile([P, 8, DPC], F32, tag="wT_sb")
            for t in range(8):
                wT_ps = paux.tile([P, P], F32, tag="aux", bufs=2, name=f"wT{t}")
                nc.tensor.transpose(
                    wT_ps[:], wnorm[:, t * P : (t + 1) * P], ident_sb[:]
                )
                nc.vector.tensor_copy(wT_sb[:, t, :], wT_ps[:])
            ctx_ps = paux.tile([P, H], F32, tag="aux", bufs=2, name="ctx_ps")  # shares aux slots
            for t in range(8):
                nc.tensor.matmul(
                    ctx_ps[:],
                    wT_sb[:, t, :],
                    enc_nat_sb[:, t, :],
                    start=(t == 0),
                    stop=(t == 7),
                )
            ctx_sb = epool.tile([P, H], F32, tag="ctx_sb")
            nc.vector.tensor_copy(ctx_sb[:], ctx_ps[:])
            nc.sync.dma_start(out=ctx_out_d[:], in_=ctx_sb[:])

    nc.compile()
    return nc


def make_in_maps(encoder_outputs, decoder_outputs, W_a, U_a, V_a):
    enc = np.ascontiguousarray(np.asarray(encoder_outputs, dtype=np.float32))
    dec = np.ascontiguousarray(np.asarray(decoder_outputs, dtype=np.float32))
    W = np.asarray(W_a, dtype=np.float32)
    U = np.asarray(U_a, dtype=np.float32)
    V = np.asarray(V_a, dtype=np.float32).reshape(H)

    enc_nat_all = enc.reshape(B, 8, P, H)  # [b, tile, p(e), h]
    encT_all = np.ascontiguousarray(enc.transpose(0, 2, 1)).reshape(B, HC, P, T_ENC)
    decT_all = np.ascontiguousarray(dec.transpose(0, 2, 1))  # [b, h, d]

    Wr = np.ascontiguousarray(W.reshape(HC, P, HC, P).transpose(0, 2, 1, 3))
    Ur = np.ascontiguousarray(U.reshape(HC, P, HC, P).transpose(0, 2, 1, 3))

    Vbig = np.zeros((P, HC, 2 * P), dtype=ml_dtypes.bfloat16)
    for c in range(HC):
        Vbig[:, c, P] = V[c * P : (c + 1) * P].astype(ml_dtypes.bfloat16)
    ident = np.eye(P, dtype=np.float32)

    in_maps = []
    for core in range(N_CORES):
        b, half = core // 2, core % 2
        dlo = half * DPC
        decT_core = np.ascontiguousarray(
            decT_all[b].reshape(HC, P, T_DEC)[:, :, dlo : dlo + DPC]
        )
        in_maps.append(
            {
                "enc_nat": enc_nat_all[b],
                "encT": encT_all[b],
                "decT": decT_core,
                "W": Wr,
                "U": Ur,
                "Vbig": Vbig,
                "ident": ident,
            }
        )
    return in_maps


def kernel(encoder_outputs, decoder_outputs, W_a, U_a, V_a):
    from concourse.bass_utils import run_bass_kernel_spmd

    in_maps = make_in_maps(encoder_outputs, decoder_outputs, W_a, U_a, V_a)
    nc = build_graph()
    res = run_bass_kernel_spmd(nc, in_maps, core_ids=list(range(N_CORES)))

    ctx = np.zeros((B, T_DEC, H), dtype=np.float32)
    w = np.zeros((B, T_DEC, T_ENC), dtype=np.float32)
    for core in range(N_CORES):
        b, half = core // 2, core % 2
        dlo = half * DPC
        out = res.results[core]
        ctx[b, dlo : dlo + DPC] = out["ctx_out"]
        w[b, dlo : dlo + DPC] = out["w_out"]
    return ctx, w
